# revision 53
# baseline (speedup 1.0000x reference)
"""DCN block kernel for Trainium2 (8 NeuronCores, data-parallel over batch).

Math (per batch b, plane c), with z = conv3x3(x, w_off) + b_off:
  delta     = (sigmoid(z) - 0.5) * 2 = tanh(z/2)     (2 offset maps per plane)
  pixel displacement d = delta / 2, |d| < 0.5  =>  bilinear has 3x3 support.
Per axis the bilinear gather is evaluated gather-free in relu-weight form:
  H(x row)  = x + relu(d)*(x(c-1)-x(c)) + relu(-d)*(x(c+1)-x(c))
  out       = H0 + relu(dy)*(Hm-H0) + relu(-dy)*(Hp-H0)
with reflection handled exactly by loading a reflect-padded copy of x for the
sampling stage (and a zero-padded copy for the convs).  relu(+-0.5*t) =
0.5*relu(+-t) folds the /2 into the activation scale for free.

Layout: 4 image row-quarters stacked on partition groups [4 x 32ch].  Convs
run as flat-geometry matmuls over 512-wide chunks (taps are flat offsets
kh*WP+kw-1), 4 concurrent tile_position streams, accumulating in PSUM.
Engine split per slab: PE convs | ACT tanh + relu weight maps + out bias |
DVE interpolation products/sums | GPSIMD column diffs + fold DMAs.
"""

from contextlib import ExitStack

import ml_dtypes
import numpy as np

import concourse.bacc as bacc
import concourse.bass as bass
import concourse.mybir as mybir
import concourse.tile as tile

BF16 = mybir.dt.bfloat16
F32 = mybir.dt.float32
AF = mybir.ActivationFunctionType
OP = mybir.AluOpType

N_CORES = 8
C = 32          # input/output channels per plane set
OC2 = 64        # offset logits (2 per plane)


class Cfg:
    def __init__(self, H=384, nr=8):
        self.H = H
        self.W = H
        self.WP = self.W + 2          # padded row: [pad, 0..W-1, pad]
        self.QH = H // 4              # rows per quarter
        assert self.QH % nr == 0
        self.nr = nr                  # output rows per quarter per slab
        self.nslab = self.QH // nr


def _f(ap):
    return ap.rearrange("p a b -> p (a b)")


def build_nc(cfg: Cfg, finalize=True):
    nc = bacc.Bacc()
    H, W, WP, nr = cfg.H, cfg.W, cfg.WP, cfg.nr
    QH = cfg.QH
    nx = nr + 4
    # tapered slab schedule: small slabs at both ends shorten pipeline fill
    # and drain; steady state keeps full nr-row slabs
    taper = [2, 2, 4]
    slabs = []
    r = 0
    for t in taper:
        slabs.append((r, t))
        r += t
    while r < QH - sum(taper):
        slabs.append((r, nr))
        r += nr
    for t in reversed(taper):
        slabs.append((r, t))
        r += t
    assert r == QH

    xc_in = nc.declare_dram_parameter("xc", [C, H + 4, WP], BF16, isOutput=False)
    xs_in = nc.declare_dram_parameter("xs", [C, H + 4, WP], BF16, isOutput=False)
    woff_in = nc.declare_dram_parameter("woff", [128, 9 * 256], BF16, isOutput=False)
    wdcn_in = nc.declare_dram_parameter("wdcn", [128, 9 * 128], BF16, isOutput=False)
    boff_in = nc.declare_dram_parameter("boff", [128, 1], F32, isOutput=False)
    bdcn_in = nc.declare_dram_parameter("bdcn", [128, 1], F32, isOutput=False)
    y_out = nc.declare_dram_parameter("y", [C, H, W], BF16, isOutput=True)

    with tile.TileContext(nc) as tc, ExitStack() as ctx:
        consts = ctx.enter_context(tc.tile_pool(name="consts", bufs=1))
        xcpool = ctx.enter_context(tc.tile_pool(name="xcp", bufs=2))
        xspool = ctx.enter_context(tc.tile_pool(name="xsp", bufs=2))
        sppool = ctx.enter_context(tc.tile_pool(name="spp", bufs=2))
        sxypool = ctx.enter_context(tc.tile_pool(name="sxyp", bufs=1))
        axpool = ctx.enter_context(tc.tile_pool(name="axp", bufs=2))
        wmpool = ctx.enter_context(tc.tile_pool(name="wmp", bufs=1))
        dlpool = ctx.enter_context(tc.tile_pool(name="dlp", bufs=1))
        hpool = ctx.enter_context(tc.tile_pool(name="hp", bufs=1))
        ospool = ctx.enter_context(tc.tile_pool(name="osp", bufs=2))
        ocpool = ctx.enter_context(tc.tile_pool(name="ocp", bufs=1))
        zpool = ctx.enter_context(tc.tile_pool(name="zp", bufs=3, space="PSUM"))
        z2pool = ctx.enter_context(tc.tile_pool(name="z2p", bufs=3, space="PSUM"))
        opool = ctx.enter_context(tc.tile_pool(name="op", bufs=2, space="PSUM"))

        # full-width block-structured lhsT: zeros mask cross-quarter terms, so
        # one 128-contraction matmul covers all 4 row-quarters per tap
        WOFF = consts.tile([128, 9, 256], BF16)
        nc.sync.dma_start(out=_f(WOFF), in_=woff_in[:])
        WDCN = consts.tile([128, 9, 128], BF16)
        nc.sync.dma_start(out=_f(WDCN), in_=wdcn_in[:])
        BOFF = consts.tile([128, 1], F32)
        nc.sync.dma_start(out=BOFF[:], in_=boff_in[:])
        BDCN = consts.tile([128, 1], F32)
        nc.sync.dma_start(out=BDCN[:], in_=bdcn_in[:])

        # conv_off chunking over z flat [1, Lh), conv_dcn over out flat [1, Lo)
        # chunk a flat range [1, L-1): the first/last flat elements (pad cols)
        # are excluded, their outermost tap would read past the source tile
        def chunks(L):
            out = []
            s = 1
            while s < L - 1:
                out.append((s, min(512, L - 1 - s)))
                s += 512
            return out

        state = {}

        def produce(it):
            """loads, conv_off, tanh, fold, relu weight maps for slab it."""
            r0, nri = slabs[it]
            nhi, nxi = nri + 2, nri + 4
            XC = xcpool.tile([128, nxi, WP], BF16, tag="xc", name="xc")
            XS = xspool.tile([128, nxi, WP], BF16, tag="xs", name="xs")
            for g in range(4):
                i0 = QH * g + r0
                nc.sync.dma_start(out=XC[32 * g:32 * g + 32],
                                  in_=xc_in[:, i0:i0 + nxi, :])
                nc.sync.dma_start(out=XS[32 * g:32 * g + 32],
                                  in_=xs_in[:, i0:i0 + nxi, :])
            XCf = _f(XC[:])

            SP = [sppool.tile([128, nhi, WP], BF16, tag=f"sp{p}", name=f"sp{p}")
                  for p in range(2)]
            for (s, L) in chunks(nhi * WP):
                zts = [zpool.tile([128, 512], F32, tag="z0", name="z0"),
                       z2pool.tile([128, 512], F32, tag="z1", name="z1")]
                for t in range(9):
                    kh, kw = t // 3, t % 3
                    base = s + kh * WP + kw - 1
                    for p in range(2):
                        nc.tensor.matmul(
                            zts[p][:, 0:L],
                            lhsT=WOFF[:, t, 128 * p:128 * p + 128],
                            rhs=XCf[:, base:base + L],
                            start=(t == 0), stop=(t == 8))
                for p in range(2):
                    nc.scalar.activation(
                        out=_f(SP[p])[:, s:s + L], in_=zts[p][:, 0:L],
                        func=AF.Tanh, bias=BOFF[:], scale=0.5)

            SX = sxypool.tile([128, nhi, WP], BF16, tag="sx", name="sx")
            SY = sxypool.tile([128, nhi, WP], BF16, tag="sy", name="sy")
            for g in range(4):
                p, gq = g // 2, g % 2
                nc.gpsimd.dma_start(
                    out=_f(SX[32 * g:32 * g + 32]),
                    in_=_f(SP[p])[64 * gq:64 * gq + 32, :])
                nc.gpsimd.dma_start(
                    out=_f(SY[32 * g:32 * g + 32]),
                    in_=_f(SP[p])[64 * gq + 32:64 * gq + 64, :])

            # relu weight maps on ACT (x0.5 folds the pixel displacement)
            AX = axpool.tile([128, nhi, WP], BF16, tag="ax", name="ax")
            BX = axpool.tile([128, nhi, WP], BF16, tag="bx", name="bx")
            WM = wmpool.tile([128, nhi, WP], BF16, tag="wm", name="wm")
            WPt = wmpool.tile([128, nhi, WP], BF16, tag="wpv", name="wpv")
            nc.scalar.activation(out=_f(AX), in_=_f(SX), func=AF.Relu, scale=0.5)
            nc.scalar.activation(out=_f(BX), in_=_f(SX), func=AF.Relu, scale=-0.5)
            nc.scalar.activation(out=_f(WM), in_=_f(SY), func=AF.Relu, scale=0.5)
            nc.scalar.activation(out=_f(WPt), in_=_f(SY), func=AF.Relu, scale=-0.5)
            state[it] = (XS, AX, BX, WM, WPt)

        def consume(it):
            """interpolation (DVE), conv_dcn, bias act, store for slab it."""
            r0, nri = slabs[it]
            nhi, nxi = nri + 2, nri + 4
            Lhi, Lxi = nhi * WP, nxi * WP
            XS, AX, BX, WM, WPt = state.pop(it)
            XSf = _f(XS[:])
            AXf, BXf, WMf, WPf = _f(AX), _f(BX), _f(WM), _f(WPt)

            DL = dlpool.tile([128, nxi, WP], BF16, tag="dl", name="dl")
            DR = dlpool.tile([128, nxi, WP], BF16, tag="dr", name="dr")
            DLf, DRf = _f(DL), _f(DR)
            nc.vector.tensor_tensor(
                DLf[:, 1:Lxi], XSf[:, 0:Lxi - 1], XSf[:, 1:Lxi], OP.subtract)
            nc.vector.tensor_tensor(
                DRf[:, 0:Lxi - 1], XSf[:, 1:Lxi], XSf[:, 0:Lxi - 1], OP.subtract)
            nc.vector.memset(DLf[:, 0:1], 0.0)
            nc.vector.memset(DRf[:, Lxi - 1:Lxi], 0.0)

            # horizontal interps H(-1), H(0), H(+1); OS doubles as scratch
            OS = ospool.tile([128, nhi, WP], BF16, tag="os", name="os")
            OSf = _f(OS)
            Hs = []
            for dr in (-1, 0, 1):
                o = (1 + dr) * WP
                Hd = hpool.tile([128, nhi, WP], BF16, tag=f"h{dr}", name=f"h{dr}")
                Hdf = _f(Hd)
                nc.vector.tensor_tensor(Hdf, AXf, DLf[:, o:o + Lhi], OP.mult)
                nc.vector.tensor_tensor(OSf, BXf, DRf[:, o:o + Lhi], OP.mult)
                nc.vector.tensor_tensor(Hdf, Hdf, XSf[:, o:o + Lhi], OP.add)
                nc.vector.tensor_tensor(Hdf, Hdf, OSf, OP.add)
                Hs.append(Hdf)
            Hmf, H0f, Hpf = Hs

            # vertical combine
            nc.vector.tensor_tensor(Hmf, Hmf, H0f, OP.subtract)
            nc.vector.tensor_tensor(Hpf, Hpf, H0f, OP.subtract)
            nc.vector.tensor_tensor(Hmf, WMf, Hmf, OP.mult)
            nc.vector.tensor_tensor(Hpf, WPf, Hpf, OP.mult)
            nc.vector.tensor_tensor(OSf, H0f, Hmf, OP.add)
            nc.vector.tensor_tensor(OSf, OSf, Hpf, OP.add)
            # conv_dcn zero padding: pad cols, pad rows at image top/bottom
            nc.vector.memset(OS[:, :, 0:WP:W + 1], 0.0)
            if r0 == 0:
                nc.vector.memset(_f(OS[0:32, 0:1, :]), 0.0)
            if r0 + nri == QH:
                nc.vector.memset(_f(OS[96:128, nri + 1:nri + 2, :]), 0.0)

            OCt = ocpool.tile([128, nri, WP], BF16, tag="oc", name="oc")
            OCf = _f(OCt)
            for (s, L) in chunks(nri * WP):
                ot = opool.tile([128, 512], F32, tag="ot", name="ot")
                for t in range(9):
                    kh, kw = t // 3, t % 3
                    base = s + kh * WP + kw - 1
                    nc.tensor.matmul(
                        ot[:, 0:L],
                        lhsT=WDCN[:, t, :],
                        rhs=OSf[:, base:base + L],
                        start=(t == 0), stop=(t == 8))
                nc.scalar.activation(
                    out=OCf[:, s:s + L], in_=ot[:, 0:L],
                    func=AF.Identity, bias=BDCN[:], scale=1.0)
            for g in range(4):
                rr = QH * g + r0
                nc.sync.dma_start(
                    out=y_out[:, rr:rr + nri, :],
                    in_=OCt[32 * g:32 * g + 32, :, 1:W + 1])

        # 2-deep software pipeline: fold+relu of slab i+1 and conv_off of slab
        # i+2 both hide under slab i's DVE stage
        for it in range(len(slabs) + 2):
            if it < len(slabs):
                produce(it)
            if it >= 2:
                consume(it - 2)
    if finalize:
        nc.finalize()
    return nc


def prep_weights(w_off, b_off, w_dcn, b_dcn):
    """Host-side packing into full-width block-structured lhsT tiles."""
    perm = np.concatenate([np.arange(0, 2 * C, 2), np.arange(1, 2 * C, 2)])
    wo = w_off[perm].astype(np.float32)                # [64, C, 3, 3]
    wo = wo.transpose(1, 2, 3, 0).reshape(C, 9, OC2)   # [ci, tap, m]
    # woff[k, t, 128p + m]: pair p covers quarters g=2p+gq; quarter g input
    # rows k in [32g, 32g+32) feed output cols [64gq, 64gq+64)
    woff = np.zeros((128, 9, 256), np.float32)
    for p in range(2):
        for gq in range(2):
            g = 2 * p + gq
            woff[32 * g:32 * g + 32, :, 128 * p + 64 * gq:128 * p + 64 * gq + 64] = wo
    woff = woff.reshape(128, 9 * 256)
    wd = w_dcn.astype(np.float32).transpose(1, 2, 3, 0).reshape(C, 9, C)
    wdcn = np.zeros((128, 9, 128), np.float32)
    for g in range(4):
        wdcn[32 * g:32 * g + 32, :, 32 * g:32 * g + 32] = wd
    wdcn = wdcn.reshape(128, 9 * 128)
    # tanh(z/2) with PSUM holding bias-free conv: bias slot gets b_off/2
    boff = np.tile(b_off[perm].astype(np.float32) * 0.5, 2).reshape(128, 1)
    bdcn = np.tile(b_dcn.astype(np.float32), 4).reshape(128, 1)
    return {
        "woff": woff.astype(ml_dtypes.bfloat16),
        "wdcn": wdcn.astype(ml_dtypes.bfloat16),
        "boff": boff.astype(np.float32),
        "bdcn": bdcn.astype(np.float32),
    }


_NC_CACHE = {}


def _get_nc(cfg_key):
    if cfg_key not in _NC_CACHE:
        _NC_CACHE[cfg_key] = build_nc(Cfg(H=cfg_key[0], nr=cfg_key[1]))
    return _NC_CACHE[cfg_key]


def _run(x, w_off, b_off, w_dcn, b_dcn, **spmd_kwargs):
    from concourse.bass_utils import run_bass_kernel_spmd

    B = x.shape[0]
    H = x.shape[2]
    assert x.shape == (B, C, H, H) and B == N_CORES
    nc = _get_nc((H, 8))
    w = prep_weights(np.asarray(w_off), np.asarray(b_off),
                     np.asarray(w_dcn), np.asarray(b_dcn))
    in_maps = []
    for b in range(B):
        m = dict(w)
        xb = np.asarray(x[b]).astype(ml_dtypes.bfloat16)
        m["xc"] = np.pad(xb, ((0, 0), (2, 2), (1, 1)))
        m["xs"] = np.pad(xb, ((0, 0), (2, 2), (1, 1)), mode="reflect")
        in_maps.append(m)
    return run_bass_kernel_spmd(nc, in_maps, list(range(N_CORES)), **spmd_kwargs)


def kernel(x, w_off, b_off, w_dcn, b_dcn):
    res = _run(x, w_off, b_off, w_dcn, b_dcn)
    out = np.stack([res.results[i]["y"] for i in range(N_CORES)], axis=0)
    return out.astype(np.float32)


# revision 55
# speedup vs baseline: 1.0617x; 1.0617x over previous
"""DCN block kernel for Trainium2 (8 NeuronCores, data-parallel over batch).

Math (per batch b, plane c), with z = conv3x3(x, w_off) + b_off:
  delta     = (sigmoid(z) - 0.5) * 2 = tanh(z/2)     (2 offset maps per plane)
  pixel displacement d = delta / 2, |d| < 0.5  =>  bilinear has 3x3 support.
Per axis the bilinear gather is evaluated gather-free in relu-weight form:
  H(x row)  = x + relu(d)*(x(c-1)-x(c)) + relu(-d)*(x(c+1)-x(c))
  out       = H0 + relu(dy)*(Hm-H0) + relu(-dy)*(Hp-H0)
with reflection handled exactly by loading a reflect-padded copy of x for the
sampling stage (and a zero-padded copy for the convs).  relu(+-0.5*t) =
0.5*relu(+-t) folds the /2 into the activation scale for free.

Layout: 4 image row-quarters stacked on partition groups [4 x 32ch].  Convs
run as flat-geometry matmuls over 512-wide chunks (taps are flat offsets
kh*WP+kw-1), 4 concurrent tile_position streams, accumulating in PSUM.
Engine split per slab: PE convs | ACT tanh + relu weight maps + out bias |
DVE interpolation products/sums | GPSIMD column diffs + fold DMAs.
"""

from contextlib import ExitStack

import ml_dtypes
import numpy as np

import concourse.bacc as bacc
import concourse.bass as bass
import concourse.mybir as mybir
import concourse.tile as tile

BF16 = mybir.dt.bfloat16
F32 = mybir.dt.float32
AF = mybir.ActivationFunctionType
OP = mybir.AluOpType

N_CORES = 8
C = 32          # input/output channels per plane set
OC2 = 64        # offset logits (2 per plane)


class Cfg:
    def __init__(self, H=384, nr=8):
        self.H = H
        self.W = H
        self.WP = self.W + 2          # padded row: [pad, 0..W-1, pad]
        self.QH = H // 4              # rows per quarter
        assert self.QH % nr == 0
        self.nr = nr                  # output rows per quarter per slab
        self.nslab = self.QH // nr


def _f(ap):
    return ap.rearrange("p a b -> p (a b)")


def build_nc(cfg: Cfg, finalize=True):
    nc = bacc.Bacc()
    H, W, WP, nr = cfg.H, cfg.W, cfg.WP, cfg.nr
    QH = cfg.QH
    nx = nr + 4
    # tapered slab schedule: half-size slabs at both ends shorten pipeline
    # fill and drain; steady state keeps full nr-row slabs
    hr = nr // 2
    slabs = [(0, hr), (hr, hr)]
    r = 2 * hr
    while r < QH - 2 * hr:
        slabs.append((r, nr))
        r += nr
    slabs += [(QH - 2 * hr, hr), (QH - hr, hr)]

    xc_in = nc.declare_dram_parameter("xc", [C, H + 4, WP], BF16, isOutput=False)
    xs_in = nc.declare_dram_parameter("xs", [C, H + 4, WP], BF16, isOutput=False)
    woff_in = nc.declare_dram_parameter("woff", [128, 9 * 256], BF16, isOutput=False)
    wdcn_in = nc.declare_dram_parameter("wdcn", [128, 9 * 128], BF16, isOutput=False)
    boff_in = nc.declare_dram_parameter("boff", [128, 1], F32, isOutput=False)
    bdcn_in = nc.declare_dram_parameter("bdcn", [128, 1], F32, isOutput=False)
    y_out = nc.declare_dram_parameter("y", [C, H, W], BF16, isOutput=True)

    with tile.TileContext(nc) as tc, ExitStack() as ctx:
        consts = ctx.enter_context(tc.tile_pool(name="consts", bufs=1))
        xcpool = ctx.enter_context(tc.tile_pool(name="xcp", bufs=2))
        xspool = ctx.enter_context(tc.tile_pool(name="xsp", bufs=2))
        sppool = ctx.enter_context(tc.tile_pool(name="spp", bufs=2))
        sxypool = ctx.enter_context(tc.tile_pool(name="sxyp", bufs=1))
        axpool = ctx.enter_context(tc.tile_pool(name="axp", bufs=2))
        wmpool = ctx.enter_context(tc.tile_pool(name="wmp", bufs=1))
        dlpool = ctx.enter_context(tc.tile_pool(name="dlp", bufs=1))
        hpool = ctx.enter_context(tc.tile_pool(name="hp", bufs=1))
        ospool = ctx.enter_context(tc.tile_pool(name="osp", bufs=2))
        ocpool = ctx.enter_context(tc.tile_pool(name="ocp", bufs=1))
        zpool = ctx.enter_context(tc.tile_pool(name="zp", bufs=3, space="PSUM"))
        z2pool = ctx.enter_context(tc.tile_pool(name="z2p", bufs=3, space="PSUM"))
        opool = ctx.enter_context(tc.tile_pool(name="op", bufs=2, space="PSUM"))

        # full-width block-structured lhsT: zeros mask cross-quarter terms, so
        # one 128-contraction matmul covers all 4 row-quarters per tap
        WOFF = consts.tile([128, 9, 256], BF16)
        nc.sync.dma_start(out=_f(WOFF), in_=woff_in[:])
        WDCN = consts.tile([128, 9, 128], BF16)
        nc.sync.dma_start(out=_f(WDCN), in_=wdcn_in[:])
        BOFF = consts.tile([128, 1], F32)
        nc.sync.dma_start(out=BOFF[:], in_=boff_in[:])
        BDCN = consts.tile([128, 1], F32)
        nc.sync.dma_start(out=BDCN[:], in_=bdcn_in[:])

        # conv_off chunking over z flat [1, Lh), conv_dcn over out flat [1, Lo)
        # chunk a flat range [1, L-1): the first/last flat elements (pad cols)
        # are excluded, their outermost tap would read past the source tile
        def chunks(L):
            out = []
            s = 1
            while s < L - 1:
                out.append((s, min(512, L - 1 - s)))
                s += 512
            return out

        state = {}

        def produce(it):
            """loads, conv_off, tanh, fold, relu weight maps for slab it."""
            r0, nri = slabs[it]
            nhi, nxi = nri + 2, nri + 4
            XC = xcpool.tile([128, nxi, WP], BF16, tag="xc", name="xc")
            XS = xspool.tile([128, nxi, WP], BF16, tag="xs", name="xs")
            for g in range(4):
                i0 = QH * g + r0
                nc.sync.dma_start(out=XC[32 * g:32 * g + 32],
                                  in_=xc_in[:, i0:i0 + nxi, :])
                nc.sync.dma_start(out=XS[32 * g:32 * g + 32],
                                  in_=xs_in[:, i0:i0 + nxi, :])
            XCf = _f(XC[:])

            SP = [sppool.tile([128, nhi, WP], BF16, tag=f"sp{p}", name=f"sp{p}")
                  for p in range(2)]
            for (s, L) in chunks(nhi * WP):
                zts = [zpool.tile([128, 512], F32, tag="z0", name="z0"),
                       z2pool.tile([128, 512], F32, tag="z1", name="z1")]
                for t in range(9):
                    kh, kw = t // 3, t % 3
                    base = s + kh * WP + kw - 1
                    for p in range(2):
                        nc.tensor.matmul(
                            zts[p][:, 0:L],
                            lhsT=WOFF[:, t, 128 * p:128 * p + 128],
                            rhs=XCf[:, base:base + L],
                            start=(t == 0), stop=(t == 8))
                for p in range(2):
                    nc.scalar.activation(
                        out=_f(SP[p])[:, s:s + L], in_=zts[p][:, 0:L],
                        func=AF.Tanh, bias=BOFF[:], scale=0.5)

            SX = sxypool.tile([128, nhi, WP], BF16, tag="sx", name="sx")
            SY = sxypool.tile([128, nhi, WP], BF16, tag="sy", name="sy")
            for g in range(4):
                p, gq = g // 2, g % 2
                nc.gpsimd.dma_start(
                    out=_f(SX[32 * g:32 * g + 32]),
                    in_=_f(SP[p])[64 * gq:64 * gq + 32, :])
                nc.gpsimd.dma_start(
                    out=_f(SY[32 * g:32 * g + 32]),
                    in_=_f(SP[p])[64 * gq + 32:64 * gq + 64, :])

            # relu weight maps on ACT (x0.5 folds the pixel displacement)
            AX = axpool.tile([128, nhi, WP], BF16, tag="ax", name="ax")
            BX = axpool.tile([128, nhi, WP], BF16, tag="bx", name="bx")
            WM = wmpool.tile([128, nhi, WP], BF16, tag="wm", name="wm")
            WPt = wmpool.tile([128, nhi, WP], BF16, tag="wpv", name="wpv")
            nc.scalar.activation(out=_f(AX), in_=_f(SX), func=AF.Relu, scale=0.5)
            nc.scalar.activation(out=_f(BX), in_=_f(SX), func=AF.Relu, scale=-0.5)
            nc.scalar.activation(out=_f(WM), in_=_f(SY), func=AF.Relu, scale=0.5)
            nc.scalar.activation(out=_f(WPt), in_=_f(SY), func=AF.Relu, scale=-0.5)
            state[it] = (XS, AX, BX, WM, WPt)

        def consume(it):
            """interpolation (DVE), conv_dcn, bias act, store for slab it."""
            r0, nri = slabs[it]
            nhi, nxi = nri + 2, nri + 4
            Lhi, Lxi = nhi * WP, nxi * WP
            XS, AX, BX, WM, WPt = state.pop(it)
            XSf = _f(XS[:])
            AXf, BXf, WMf, WPf = _f(AX), _f(BX), _f(WM), _f(WPt)

            # left diff DL[r,c] = x[r,c-1]-x[r,c]; the right diff is its
            # shifted negation DR[r,c] = -DL[r,c+1], read as a +1 view below
            DL = dlpool.tile([128, nxi, WP], BF16, tag="dl", name="dl")
            DLf = _f(DL)
            nc.vector.tensor_tensor(
                DLf[:, 1:Lxi], XSf[:, 0:Lxi - 1], XSf[:, 1:Lxi], OP.subtract)
            nc.vector.memset(DLf[:, 0:1], 0.0)

            # horizontal interps H(-1), H(0), H(+1); OS doubles as scratch
            OS = ospool.tile([128, nhi, WP], BF16, tag="os", name="os")
            OSf = _f(OS)
            Hs = []
            for dr in (-1, 0, 1):
                o = (1 + dr) * WP
                Lq = Lhi if o + 1 + Lhi <= Lxi else Lhi - 1  # last elem = pad col
                Hd = hpool.tile([128, nhi, WP], BF16, tag=f"h{dr}", name=f"h{dr}")
                Hdf = _f(Hd)
                nc.vector.tensor_tensor(Hdf, AXf, DLf[:, o:o + Lhi], OP.mult)
                nc.vector.tensor_tensor(OSf[:, 0:Lq], BXf[:, 0:Lq],
                                        DLf[:, o + 1:o + 1 + Lq], OP.mult)
                nc.vector.tensor_tensor(Hdf, Hdf, XSf[:, o:o + Lhi], OP.add)
                nc.vector.tensor_tensor(Hdf[:, 0:Lq], Hdf[:, 0:Lq],
                                        OSf[:, 0:Lq], OP.subtract)
                Hs.append(Hdf)
            Hmf, H0f, Hpf = Hs

            # vertical combine
            nc.vector.tensor_tensor(Hmf, Hmf, H0f, OP.subtract)
            nc.vector.tensor_tensor(Hpf, Hpf, H0f, OP.subtract)
            nc.vector.tensor_tensor(Hmf, WMf, Hmf, OP.mult)
            nc.vector.tensor_tensor(Hpf, WPf, Hpf, OP.mult)
            nc.vector.tensor_tensor(OSf, H0f, Hmf, OP.add)
            nc.vector.tensor_tensor(OSf, OSf, Hpf, OP.add)
            # conv_dcn zero padding: pad cols, pad rows at image top/bottom
            nc.vector.memset(OS[:, :, 0:WP:W + 1], 0.0)
            if r0 == 0:
                nc.vector.memset(_f(OS[0:32, 0:1, :]), 0.0)
            if r0 + nri == QH:
                nc.vector.memset(_f(OS[96:128, nri + 1:nri + 2, :]), 0.0)

            OCt = ocpool.tile([128, nri, WP], BF16, tag="oc", name="oc")
            OCf = _f(OCt)
            for (s, L) in chunks(nri * WP):
                ot = opool.tile([128, 512], F32, tag="ot", name="ot")
                for t in range(9):
                    kh, kw = t // 3, t % 3
                    base = s + kh * WP + kw - 1
                    nc.tensor.matmul(
                        ot[:, 0:L],
                        lhsT=WDCN[:, t, :],
                        rhs=OSf[:, base:base + L],
                        start=(t == 0), stop=(t == 8))
                nc.scalar.activation(
                    out=OCf[:, s:s + L], in_=ot[:, 0:L],
                    func=AF.Identity, bias=BDCN[:], scale=1.0)
            for g in range(4):
                rr = QH * g + r0
                nc.sync.dma_start(
                    out=y_out[:, rr:rr + nri, :],
                    in_=OCt[32 * g:32 * g + 32, :, 1:W + 1])

        # 2-deep software pipeline: fold+relu of slab i+1 and conv_off of slab
        # i+2 both hide under slab i's DVE stage
        for it in range(len(slabs) + 2):
            if it < len(slabs):
                produce(it)
            if it >= 2:
                consume(it - 2)
    if finalize:
        nc.finalize()
    return nc


def prep_weights(w_off, b_off, w_dcn, b_dcn):
    """Host-side packing into full-width block-structured lhsT tiles."""
    perm = np.concatenate([np.arange(0, 2 * C, 2), np.arange(1, 2 * C, 2)])
    wo = w_off[perm].astype(np.float32)                # [64, C, 3, 3]
    wo = wo.transpose(1, 2, 3, 0).reshape(C, 9, OC2)   # [ci, tap, m]
    # woff[k, t, 128p + m]: pair p covers quarters g=2p+gq; quarter g input
    # rows k in [32g, 32g+32) feed output cols [64gq, 64gq+64)
    woff = np.zeros((128, 9, 256), np.float32)
    for p in range(2):
        for gq in range(2):
            g = 2 * p + gq
            woff[32 * g:32 * g + 32, :, 128 * p + 64 * gq:128 * p + 64 * gq + 64] = wo
    woff = woff.reshape(128, 9 * 256)
    wd = w_dcn.astype(np.float32).transpose(1, 2, 3, 0).reshape(C, 9, C)
    wdcn = np.zeros((128, 9, 128), np.float32)
    for g in range(4):
        wdcn[32 * g:32 * g + 32, :, 32 * g:32 * g + 32] = wd
    wdcn = wdcn.reshape(128, 9 * 128)
    # tanh(z/2) with PSUM holding bias-free conv: bias slot gets b_off/2
    boff = np.tile(b_off[perm].astype(np.float32) * 0.5, 2).reshape(128, 1)
    bdcn = np.tile(b_dcn.astype(np.float32), 4).reshape(128, 1)
    return {
        "woff": woff.astype(ml_dtypes.bfloat16),
        "wdcn": wdcn.astype(ml_dtypes.bfloat16),
        "boff": boff.astype(np.float32),
        "bdcn": bdcn.astype(np.float32),
    }


_NC_CACHE = {}


def _get_nc(cfg_key):
    if cfg_key not in _NC_CACHE:
        _NC_CACHE[cfg_key] = build_nc(Cfg(H=cfg_key[0], nr=cfg_key[1]))
    return _NC_CACHE[cfg_key]


def _run(x, w_off, b_off, w_dcn, b_dcn, **spmd_kwargs):
    from concourse.bass_utils import run_bass_kernel_spmd

    B = x.shape[0]
    H = x.shape[2]
    assert x.shape == (B, C, H, H) and B == N_CORES
    nc = _get_nc((H, 8))
    w = prep_weights(np.asarray(w_off), np.asarray(b_off),
                     np.asarray(w_dcn), np.asarray(b_dcn))
    in_maps = []
    for b in range(B):
        m = dict(w)
        xb = np.asarray(x[b]).astype(ml_dtypes.bfloat16)
        m["xc"] = np.pad(xb, ((0, 0), (2, 2), (1, 1)))
        m["xs"] = np.pad(xb, ((0, 0), (2, 2), (1, 1)), mode="reflect")
        in_maps.append(m)
    return run_bass_kernel_spmd(nc, in_maps, list(range(N_CORES)), **spmd_kwargs)


def kernel(x, w_off, b_off, w_dcn, b_dcn):
    res = _run(x, w_off, b_off, w_dcn, b_dcn)
    out = np.stack([res.results[i]["y"] for i in range(N_CORES)], axis=0)
    return out.astype(np.float32)


# revision 56
# speedup vs baseline: 1.0873x; 1.0241x over previous
"""DCN block kernel for Trainium2 (8 NeuronCores, data-parallel over batch).

Math (per batch b, plane c), with z = conv3x3(x, w_off) + b_off:
  delta     = (sigmoid(z) - 0.5) * 2 = tanh(z/2)     (2 offset maps per plane)
  pixel displacement d = delta / 2, |d| < 0.5  =>  bilinear has 3x3 support.
Per axis the bilinear gather is evaluated gather-free in relu-weight form:
  H(x row)  = x + relu(d)*(x(c-1)-x(c)) + relu(-d)*(x(c+1)-x(c))
  out       = H0 + relu(dy)*(Hm-H0) + relu(-dy)*(Hp-H0)
with reflection handled exactly by loading a reflect-padded copy of x for the
sampling stage (and a zero-padded copy for the convs).  relu(+-0.5*t) =
0.5*relu(+-t) folds the /2 into the activation scale for free.

Layout: 4 image row-quarters stacked on partition groups [4 x 32ch].  Convs
run as flat-geometry matmuls over 512-wide chunks (taps are flat offsets
kh*WP+kw-1), 4 concurrent tile_position streams, accumulating in PSUM.
Engine split per slab: PE convs | ACT tanh + relu weight maps + out bias |
DVE interpolation products/sums | GPSIMD column diffs + fold DMAs.
"""

from contextlib import ExitStack

import ml_dtypes
import numpy as np

import concourse.bacc as bacc
import concourse.bass as bass
import concourse.mybir as mybir
import concourse.tile as tile

BF16 = mybir.dt.bfloat16
F32 = mybir.dt.float32
AF = mybir.ActivationFunctionType
OP = mybir.AluOpType

N_CORES = 8
C = 32          # input/output channels per plane set
OC2 = 64        # offset logits (2 per plane)


class Cfg:
    def __init__(self, H=384, nr=8):
        self.H = H
        self.W = H
        self.WP = self.W + 2          # padded row: [pad, 0..W-1, pad]
        self.QH = H // 4              # rows per quarter
        assert self.QH % nr == 0
        self.nr = nr                  # output rows per quarter per slab
        self.nslab = self.QH // nr


def _f(ap):
    return ap.rearrange("p a b -> p (a b)")


def build_nc(cfg: Cfg, finalize=True):
    nc = bacc.Bacc()
    H, W, WP, nr = cfg.H, cfg.W, cfg.WP, cfg.nr
    QH = cfg.QH
    nx = nr + 4
    # tapered slab schedule: half-size slabs at both ends shorten pipeline
    # fill and drain; steady state keeps full nr-row slabs
    hr = nr // 2
    slabs = [(0, hr), (hr, hr)]
    r = 2 * hr
    while r < QH - 2 * hr:
        slabs.append((r, nr))
        r += nr
    slabs += [(QH - 2 * hr, hr), (QH - hr, hr)]

    xc_in = nc.declare_dram_parameter("xc", [C, H + 4, WP], BF16, isOutput=False)
    xs_in = nc.declare_dram_parameter("xs", [C, H + 4, WP], BF16, isOutput=False)
    woff_in = nc.declare_dram_parameter("woff", [128, 9 * 256], BF16, isOutput=False)
    wdcn_in = nc.declare_dram_parameter("wdcn", [128, 9 * 128], BF16, isOutput=False)
    boff_in = nc.declare_dram_parameter("boff", [128, 1], F32, isOutput=False)
    bdcn_in = nc.declare_dram_parameter("bdcn", [128, 1], F32, isOutput=False)
    y_out = nc.declare_dram_parameter("y", [C, H, W], BF16, isOutput=True)

    with tile.TileContext(nc) as tc, ExitStack() as ctx:
        consts = ctx.enter_context(tc.tile_pool(name="consts", bufs=1))
        xcpool = ctx.enter_context(tc.tile_pool(name="xcp", bufs=2))
        xspool = ctx.enter_context(tc.tile_pool(name="xsp", bufs=2))
        sppool = ctx.enter_context(tc.tile_pool(name="spp", bufs=2))
        sxypool = ctx.enter_context(tc.tile_pool(name="sxyp", bufs=1))
        axpool = ctx.enter_context(tc.tile_pool(name="axp", bufs=2))
        wmpool = ctx.enter_context(tc.tile_pool(name="wmp", bufs=1))
        dlpool = ctx.enter_context(tc.tile_pool(name="dlp", bufs=1))
        hpool = ctx.enter_context(tc.tile_pool(name="hp", bufs=1))
        ospool = ctx.enter_context(tc.tile_pool(name="osp", bufs=2))
        ocpool = ctx.enter_context(tc.tile_pool(name="ocp", bufs=1))
        zpool = ctx.enter_context(tc.tile_pool(name="zp", bufs=3, space="PSUM"))
        z2pool = ctx.enter_context(tc.tile_pool(name="z2p", bufs=3, space="PSUM"))
        opool = ctx.enter_context(tc.tile_pool(name="op", bufs=2, space="PSUM"))

        # full-width block-structured lhsT: zeros mask cross-quarter terms, so
        # one 128-contraction matmul covers all 4 row-quarters per tap
        WOFF = consts.tile([128, 9, 256], BF16)
        nc.sync.dma_start(out=_f(WOFF), in_=woff_in[:])
        WDCN = consts.tile([128, 9, 128], BF16)
        nc.sync.dma_start(out=_f(WDCN), in_=wdcn_in[:])
        BOFF = consts.tile([128, 1], F32)
        nc.sync.dma_start(out=BOFF[:], in_=boff_in[:])
        BDCN = consts.tile([128, 1], F32)
        nc.sync.dma_start(out=BDCN[:], in_=bdcn_in[:])

        # conv_off chunking over z flat [1, Lh), conv_dcn over out flat [1, Lo)
        # chunk a flat range [1, L-1): the first/last flat elements (pad cols)
        # are excluded, their outermost tap would read past the source tile
        def chunks(L):
            out = []
            s = 1
            while s < L - 1:
                out.append((s, min(512, L - 1 - s)))
                s += 512
            return out

        state = {}

        def produce(it):
            """loads, conv_off, tanh, fold, relu weight maps for slab it.

            Slab 0 computes all nh z rows [-1, nr+1); later slabs compute only
            their nr new rows [r0+1, r0+nr+1) -- the 2 overlap rows come from
            the previous slab's still-live tanh tile at fold time."""
            r0, nri = slabs[it]
            nhi, nxi = nri + 2, nri + 4
            first = (it == 0)
            zrows = nhi if first else nri
            XC = xcpool.tile([128, zrows + 2, WP], BF16, tag="xc", name="xc")
            XS = xspool.tile([128, nxi, WP], BF16, tag="xs", name="xs")
            for g in range(4):
                i0 = QH * g + r0 + (0 if first else 2)
                nc.sync.dma_start(out=XC[32 * g:32 * g + 32],
                                  in_=xc_in[:, i0:i0 + zrows + 2, :])
                nc.sync.dma_start(out=XS[32 * g:32 * g + 32],
                                  in_=xs_in[:, QH * g + r0:QH * g + r0 + nxi, :])
            XCf = _f(XC[:])

            SP = [sppool.tile([128, zrows, WP], BF16, tag=f"sp{p}", name=f"sp{p}")
                  for p in range(2)]
            for (s, L) in chunks(zrows * WP):
                zts = [zpool.tile([128, 512], F32, tag="z0", name="z0"),
                       z2pool.tile([128, 512], F32, tag="z1", name="z1")]
                for t in range(9):
                    kh, kw = t // 3, t % 3
                    base = s + kh * WP + kw - 1
                    for p in range(2):
                        nc.tensor.matmul(
                            zts[p][:, 0:L],
                            lhsT=WOFF[:, t, 128 * p:128 * p + 128],
                            rhs=XCf[:, base:base + L],
                            start=(t == 0), stop=(t == 8))
                for p in range(2):
                    nc.scalar.activation(
                        out=_f(SP[p])[:, s:s + L], in_=zts[p][:, 0:L],
                        func=AF.Tanh, bias=BOFF[:], scale=0.5)

            SX = sxypool.tile([128, nhi, WP], BF16, tag="sx", name="sx")
            SY = sxypool.tile([128, nhi, WP], BF16, tag="sy", name="sy")
            for g in range(4):
                p, gq = g // 2, g % 2
                if first:
                    nc.gpsimd.dma_start(
                        out=_f(SX[32 * g:32 * g + 32]),
                        in_=_f(SP[p])[64 * gq:64 * gq + 32, :])
                    nc.gpsimd.dma_start(
                        out=_f(SY[32 * g:32 * g + 32]),
                        in_=_f(SP[p])[64 * gq + 32:64 * gq + 64, :])
                else:
                    prevSP, pz = state[("sp", it - 1)]
                    nc.gpsimd.dma_start(
                        out=SX[32 * g:32 * g + 32, 0:2, :],
                        in_=prevSP[p][64 * gq:64 * gq + 32, pz - 2:pz, :])
                    nc.gpsimd.dma_start(
                        out=SY[32 * g:32 * g + 32, 0:2, :],
                        in_=prevSP[p][64 * gq + 32:64 * gq + 64, pz - 2:pz, :])
                    nc.gpsimd.dma_start(
                        out=SX[32 * g:32 * g + 32, 2:nhi, :],
                        in_=SP[p][64 * gq:64 * gq + 32, :, :])
                    nc.gpsimd.dma_start(
                        out=SY[32 * g:32 * g + 32, 2:nhi, :],
                        in_=SP[p][64 * gq + 32:64 * gq + 64, :, :])
            state[("sp", it)] = (SP, zrows)

            # relu weight maps on ACT (x0.5 folds the pixel displacement)
            AX = axpool.tile([128, nhi, WP], BF16, tag="ax", name="ax")
            BX = axpool.tile([128, nhi, WP], BF16, tag="bx", name="bx")
            WM = wmpool.tile([128, nhi, WP], BF16, tag="wm", name="wm")
            WPt = wmpool.tile([128, nhi, WP], BF16, tag="wpv", name="wpv")
            nc.scalar.activation(out=_f(AX), in_=_f(SX), func=AF.Relu, scale=0.5)
            nc.scalar.activation(out=_f(BX), in_=_f(SX), func=AF.Relu, scale=-0.5)
            nc.scalar.activation(out=_f(WM), in_=_f(SY), func=AF.Relu, scale=0.5)
            nc.scalar.activation(out=_f(WPt), in_=_f(SY), func=AF.Relu, scale=-0.5)
            state[it] = (XS, AX, BX, WM, WPt)

        def consume(it):
            """interpolation (DVE), conv_dcn, bias act, store for slab it."""
            r0, nri = slabs[it]
            nhi, nxi = nri + 2, nri + 4
            Lhi, Lxi = nhi * WP, nxi * WP
            XS, AX, BX, WM, WPt = state.pop(it)
            XSf = _f(XS[:])
            AXf, BXf, WMf, WPf = _f(AX), _f(BX), _f(WM), _f(WPt)

            # left diff DL[r,c] = x[r,c-1]-x[r,c]; the right diff is its
            # shifted negation DR[r,c] = -DL[r,c+1], read as a +1 view below
            DL = dlpool.tile([128, nxi, WP], BF16, tag="dl", name="dl")
            DLf = _f(DL)
            nc.vector.tensor_tensor(
                DLf[:, 1:Lxi], XSf[:, 0:Lxi - 1], XSf[:, 1:Lxi], OP.subtract)
            nc.vector.memset(DLf[:, 0:1], 0.0)

            # horizontal interps H(-1), H(0), H(+1); OS doubles as scratch
            OS = ospool.tile([128, nhi, WP], BF16, tag="os", name="os")
            OSf = _f(OS)
            Hs = []
            for dr in (-1, 0, 1):
                o = (1 + dr) * WP
                Lq = Lhi if o + 1 + Lhi <= Lxi else Lhi - 1  # last elem = pad col
                Hd = hpool.tile([128, nhi, WP], BF16, tag=f"h{dr}", name=f"h{dr}")
                Hdf = _f(Hd)
                nc.vector.tensor_tensor(Hdf, AXf, DLf[:, o:o + Lhi], OP.mult)
                nc.vector.tensor_tensor(OSf[:, 0:Lq], BXf[:, 0:Lq],
                                        DLf[:, o + 1:o + 1 + Lq], OP.mult)
                nc.vector.tensor_tensor(Hdf, Hdf, XSf[:, o:o + Lhi], OP.add)
                nc.vector.tensor_tensor(Hdf[:, 0:Lq], Hdf[:, 0:Lq],
                                        OSf[:, 0:Lq], OP.subtract)
                Hs.append(Hdf)
            Hmf, H0f, Hpf = Hs

            # vertical combine
            nc.vector.tensor_tensor(Hmf, Hmf, H0f, OP.subtract)
            nc.vector.tensor_tensor(Hpf, Hpf, H0f, OP.subtract)
            nc.vector.tensor_tensor(Hmf, WMf, Hmf, OP.mult)
            nc.vector.tensor_tensor(Hpf, WPf, Hpf, OP.mult)
            nc.vector.tensor_tensor(OSf, H0f, Hmf, OP.add)
            nc.vector.tensor_tensor(OSf, OSf, Hpf, OP.add)
            # conv_dcn zero padding: pad cols, pad rows at image top/bottom
            nc.vector.memset(OS[:, :, 0:WP:W + 1], 0.0)
            if r0 == 0:
                nc.vector.memset(_f(OS[0:32, 0:1, :]), 0.0)
            if r0 + nri == QH:
                nc.vector.memset(_f(OS[96:128, nri + 1:nri + 2, :]), 0.0)

            OCt = ocpool.tile([128, nri, WP], BF16, tag="oc", name="oc")
            OCf = _f(OCt)
            for (s, L) in chunks(nri * WP):
                ot = opool.tile([128, 512], F32, tag="ot", name="ot")
                for t in range(9):
                    kh, kw = t // 3, t % 3
                    base = s + kh * WP + kw - 1
                    nc.tensor.matmul(
                        ot[:, 0:L],
                        lhsT=WDCN[:, t, :],
                        rhs=OSf[:, base:base + L],
                        start=(t == 0), stop=(t == 8))
                nc.scalar.activation(
                    out=OCf[:, s:s + L], in_=ot[:, 0:L],
                    func=AF.Identity, bias=BDCN[:], scale=1.0)
            for g in range(4):
                rr = QH * g + r0
                nc.sync.dma_start(
                    out=y_out[:, rr:rr + nri, :],
                    in_=OCt[32 * g:32 * g + 32, :, 1:W + 1])

        # 2-deep software pipeline: fold+relu of slab i+1 and conv_off of slab
        # i+2 both hide under slab i's DVE stage
        for it in range(len(slabs) + 2):
            if it < len(slabs):
                produce(it)
            if it >= 2:
                consume(it - 2)
    if finalize:
        nc.finalize()
    return nc


def prep_weights(w_off, b_off, w_dcn, b_dcn):
    """Host-side packing into full-width block-structured lhsT tiles."""
    perm = np.concatenate([np.arange(0, 2 * C, 2), np.arange(1, 2 * C, 2)])
    wo = w_off[perm].astype(np.float32)                # [64, C, 3, 3]
    wo = wo.transpose(1, 2, 3, 0).reshape(C, 9, OC2)   # [ci, tap, m]
    # woff[k, t, 128p + m]: pair p covers quarters g=2p+gq; quarter g input
    # rows k in [32g, 32g+32) feed output cols [64gq, 64gq+64)
    woff = np.zeros((128, 9, 256), np.float32)
    for p in range(2):
        for gq in range(2):
            g = 2 * p + gq
            woff[32 * g:32 * g + 32, :, 128 * p + 64 * gq:128 * p + 64 * gq + 64] = wo
    woff = woff.reshape(128, 9 * 256)
    wd = w_dcn.astype(np.float32).transpose(1, 2, 3, 0).reshape(C, 9, C)
    wdcn = np.zeros((128, 9, 128), np.float32)
    for g in range(4):
        wdcn[32 * g:32 * g + 32, :, 32 * g:32 * g + 32] = wd
    wdcn = wdcn.reshape(128, 9 * 128)
    # tanh(z/2) with PSUM holding bias-free conv: bias slot gets b_off/2
    boff = np.tile(b_off[perm].astype(np.float32) * 0.5, 2).reshape(128, 1)
    bdcn = np.tile(b_dcn.astype(np.float32), 4).reshape(128, 1)
    return {
        "woff": woff.astype(ml_dtypes.bfloat16),
        "wdcn": wdcn.astype(ml_dtypes.bfloat16),
        "boff": boff.astype(np.float32),
        "bdcn": bdcn.astype(np.float32),
    }


_NC_CACHE = {}


def _get_nc(cfg_key):
    if cfg_key not in _NC_CACHE:
        _NC_CACHE[cfg_key] = build_nc(Cfg(H=cfg_key[0], nr=cfg_key[1]))
    return _NC_CACHE[cfg_key]


def _run(x, w_off, b_off, w_dcn, b_dcn, **spmd_kwargs):
    from concourse.bass_utils import run_bass_kernel_spmd

    B = x.shape[0]
    H = x.shape[2]
    assert x.shape == (B, C, H, H) and B == N_CORES
    nc = _get_nc((H, 8))
    w = prep_weights(np.asarray(w_off), np.asarray(b_off),
                     np.asarray(w_dcn), np.asarray(b_dcn))
    in_maps = []
    for b in range(B):
        m = dict(w)
        xb = np.asarray(x[b]).astype(ml_dtypes.bfloat16)
        m["xc"] = np.pad(xb, ((0, 0), (2, 2), (1, 1)))
        m["xs"] = np.pad(xb, ((0, 0), (2, 2), (1, 1)), mode="reflect")
        in_maps.append(m)
    return run_bass_kernel_spmd(nc, in_maps, list(range(N_CORES)), **spmd_kwargs)


def kernel(x, w_off, b_off, w_dcn, b_dcn):
    res = _run(x, w_off, b_off, w_dcn, b_dcn)
    out = np.stack([res.results[i]["y"] for i in range(N_CORES)], axis=0)
    return out.astype(np.float32)
